# revision 1
# baseline (speedup 1.0000x reference)
"""Bipartite GNN attention kernel for Trainium2, SPMD across 8 NeuronCores.

Math (per reference):
  u = user @ W_u.T + b_u ; v = item @ W_v.T + b_v
  learn_user = softmax((u @ v.T) * UV_adj * scale, axis=1) @ v + u
  learn_item = softmax((v @ u.T) * VU_adj * scale, axis=1) @ u + v

Sharding: core i owns rows [i*1024, (i+1)*1024) of BOTH outputs; no
collectives (the contracted-side projection is replicated).

v2 design:
- Projections run in float32r (full f32 input precision, FP22 matmul) so
  the u/v terms that reach the output directly keep ~1e-4 accuracy.
- The projected feature matrices fT (feature-major, [h,*]) are stored in
  bf16: both directions' 16K-column matrices fit in SBUF at once, so the
  second direction's projection DMA stream prefetches during the first
  direction's attention loop.
- Scores/aggregation/denominator matmuls run in bf16 (same PE rate as
  f32r, FWL halves weight-load cost). Score/softmax epilogue arithmetic
  (mask multiply, exp input, division, +u) stays f32.
- Row-major v chunks come from 2-byte DMA XBAR transposes of fT instead
  of PE transposes (PE and DVE freed; DMA has headroom).
- S^T layout: scores tile is [item-chunk=128 part, user-rows=512 free];
  exp'd tile feeds aggregation directly as the stationary operand and a
  ones-vector matmul accumulates the softmax denominator.
"""

import sys

sys.path.insert(0, "/opt/trn_rl_repo")

import ml_dtypes
import numpy as np

import concourse.bacc as bacc
import concourse.bass as bass
import concourse.mybir as mybir
import concourse.tile as tile
from concourse.bass_utils import run_bass_kernel_spmd

N = 8192          # users == items
H = 512           # hidden
NCORES = 8
RB = N // NCORES  # 1024 rows per core per direction
KH = H // 128     # 4 h-chunks
NB = N // 128     # 64 column chunks
NRB = RB // 512   # 2 r-blocks of 512
SCALE = float(1.0 / np.sqrt(np.float32(H)))

F32 = mybir.dt.float32
F32R = mybir.dt.float32r
BF16 = mybir.dt.bfloat16


def _r(ap):
    return ap.bitcast(F32R)


def build_nc():
    nc = bacc.Bacc("TRN2", target_bir_lowering=False, debug=False)

    userT = nc.declare_dram_parameter("userT", [H, N], F32, isOutput=False)
    itemT = nc.declare_dram_parameter("itemT", [H, N], F32, isOutput=False)
    userT_blk = nc.declare_dram_parameter("userT_blk", [H, RB], F32, isOutput=False)
    itemT_blk = nc.declare_dram_parameter("itemT_blk", [H, RB], F32, isOutput=False)
    maskA = nc.declare_dram_parameter("maskA", [N, RB], BF16, isOutput=False)
    maskB = nc.declare_dram_parameter("maskB", [N, RB], BF16, isOutput=False)
    W_uT = nc.declare_dram_parameter("W_uT", [H, H], F32, isOutput=False)
    W_vT = nc.declare_dram_parameter("W_vT", [H, H], F32, isOutput=False)
    b_u_p = nc.declare_dram_parameter("b_u_p", [128, KH], F32, isOutput=False)
    b_v_p = nc.declare_dram_parameter("b_v_p", [128, KH], F32, isOutput=False)
    ident = nc.declare_dram_parameter("ident", [128, 128], F32, isOutput=False)
    out = nc.declare_dram_parameter("out", [2 * RB, H], F32, isOutput=True)

    with tile.TileContext(nc) as tc:
        with (
            tc.tile_pool(name="bigA", bufs=1) as bigA,
            tc.tile_pool(name="bigB", bufs=1) as bigB,
            tc.tile_pool(name="qtb", bufs=1) as qtbp,
            tc.tile_pool(name="qt32", bufs=1) as qt32p,
            tc.tile_pool(name="wts", bufs=1) as wts,
            tc.tile_pool(name="stream", bufs=12) as stream,
            tc.tile_pool(name="mask", bufs=3) as maskp,
            tc.tile_pool(name="pf", bufs=2) as pfp,
            tc.tile_pool(name="pb", bufs=3) as pbp,
            tc.tile_pool(name="vchunk", bufs=3) as vchp,
            tc.tile_pool(name="outs", bufs=1) as outsp,
            tc.tile_pool(name="small", bufs=1) as small,
            tc.tile_pool(name="ps_s", bufs=2, space="PSUM") as ps_s,      # 2 banks
            tc.tile_pool(name="ps_tr", bufs=1, space="PSUM") as ps_tr,    # 1 bank
            tc.tile_pool(name="ps_agg", bufs=1, space="PSUM") as ps_agg,  # 4 banks
            tc.tile_pool(name="ps_rs", bufs=1, space="PSUM") as ps_rs,    # 1 bank
        ):
            identity = small.tile([128, 128], F32R, tag="ident")
            nc.sync.dma_start(identity[:], ident[:].bitcast(F32R))
            identity_bf = small.tile([128, 128], BF16, tag="identbf")
            nc.vector.tensor_copy(identity_bf[:], identity[:])
            ones_bf = small.tile([128, 1], BF16, tag="ones")
            nc.vector.memset(ones_bf[:], 1.0)
            zbias = small.tile([128, 1], F32, tag="zbias")
            nc.vector.memset(zbias[:], 0.0)
            bu_sb = small.tile([128, KH], F32, tag="bu")
            nc.sync.dma_start(bu_sb[:], b_u_p[:])
            bv_sb = small.tile([128, KH], F32, tag="bv")
            nc.sync.dma_start(bv_sb[:], b_v_p[:])

            def direction(big_pool, pname, featT_dram, qT_blk_dram, w_feat_dram,
                          w_q_dram, bias_feat, bias_q, mask_dram, out_base):
                wq = [wts.tile([128, H], F32R, tag=f"w{k}", name=f"{pname}wq{k}")
                      for k in range(KH)]
                for k in range(KH):
                    for h in range(2):
                        nc.sync.dma_start(
                            wq[k][:, h * 256:(h + 1) * 256],
                            w_q_dram[k * 128:(k + 1) * 128,
                                     h * 256:(h + 1) * 256].bitcast(F32R))

                # -- project q^T block first: bf16 (scores rhs) + f32r --
                qTb = [qtbp.tile([128, RB], BF16, tag=f"qTb{m}", name=f"{pname}qTb{m}")
                       for m in range(KH)]
                qT32 = [qt32p.tile([128, RB], F32R, tag=f"qT32{m}",
                                   name=f"{pname}qT32{m}") for m in range(KH)]
                qt_in = [stream.tile([128, 512], F32R, tag="ft_in",
                                     name=f"{pname}qt{j}_{k}")
                         for j in range(2) for k in range(KH)]
                for j in range(2):
                    for k in range(KH):
                        nc.sync.dma_start(
                            qt_in[j * KH + k][:],
                            qT_blk_dram[k * 128:(k + 1) * 128,
                                        j * 512:(j + 1) * 512].bitcast(F32R))
                for m in range(KH):
                    ps0 = ps_s.tile([128, 512], F32, tag="s")
                    ps1 = ps_s.tile([128, 512], F32, tag="s")
                    for k in range(KH):
                        nc.tensor.matmul(
                            ps0[:], _r(wq[k][:, m * 128:(m + 1) * 128]),
                            qt_in[k][:], start=(k == 0), stop=(k == KH - 1))
                        nc.tensor.matmul(
                            ps1[:], _r(wq[k][:, m * 128:(m + 1) * 128]),
                            qt_in[KH + k][:], start=(k == 0), stop=(k == KH - 1))
                    for j, ps in ((0, ps0), (1, ps1)):
                        nc.vector.tensor_scalar(
                            out=qTb[m][:, j * 512:(j + 1) * 512], in0=ps[:],
                            scalar1=bias_q[:, m:m + 1], scalar2=None,
                            op0=mybir.AluOpType.add)
                        nc.scalar.add(
                            qT32[m][:, j * 512:(j + 1) * 512], ps[:],
                            bias_q[:, m:m + 1])

                wf = [wts.tile([128, H], F32R, tag=f"w{k}", name=f"{pname}wf{k}")
                      for k in range(KH)]
                for k in range(KH):
                    nc.sync.dma_start(
                        wf[k][:], w_feat_dram[k * 128:(k + 1) * 128, :].bitcast(F32R))
                # -- fT projection pairs, emitted interleaved into rb=0 --
                fT = [big_pool.tile([128, N], BF16, tag=f"{pname}fT{m}",
                                    name=f"{pname}fT{m}") for m in range(KH)]

                def emit_pair(np_):
                    ft_in = [stream.tile([128, 512], F32R, tag="ft_in",
                                         name=f"{pname}ft{np_}_{j}_{k}")
                             for j in range(2) for k in range(KH)]
                    for j in range(2):
                        for k in range(KH):
                            nc.sync.dma_start(
                                ft_in[j * KH + k][:],
                                featT_dram[k * 128:(k + 1) * 128,
                                           (2 * np_ + j) * 512:
                                           (2 * np_ + j + 1) * 512].bitcast(F32R))
                    for m in range(KH):
                        ps0 = ps_s.tile([128, 512], F32, tag="s")
                        ps1 = ps_s.tile([128, 512], F32, tag="s")
                        for k in range(KH):
                            nc.tensor.matmul(
                                ps0[:], _r(wf[k][:, m * 128:(m + 1) * 128]),
                                ft_in[k][:], start=(k == 0), stop=(k == KH - 1))
                            nc.tensor.matmul(
                                ps1[:], _r(wf[k][:, m * 128:(m + 1) * 128]),
                                ft_in[KH + k][:], start=(k == 0), stop=(k == KH - 1))
                        nc.vector.tensor_scalar(
                            out=fT[m][:, (2 * np_) * 512:(2 * np_ + 1) * 512],
                            in0=ps0[:], scalar1=bias_feat[:, m:m + 1], scalar2=None,
                            op0=mybir.AluOpType.add)
                        nc.scalar.add(
                            fT[m][:, (2 * np_ + 1) * 512:(2 * np_ + 2) * 512],
                            ps1[:], bias_feat[:, m:m + 1])

                # -- attention main loop (rb=0 carries the projection pairs) --
                for rb in range(NRB):
                    agg = ps_agg.tile([128, KH, 512], F32, tag="agg")
                    rsum4 = ps_rs.tile([128, 4], F32, tag="rs")
                    for b in range(NB):
                        if rb == 0 and b % 8 == 0:
                            emit_pair(b // 8)
                        # row-major feat chunk via PE transpose (bf16),
                        # interleaved with the score matmuls that load the
                        # same fT slice as weights
                        tp = ps_tr.tile([128, 512], BF16, tag="tr")
                        sps = ps_s.tile([128, 512], F32, tag="s")
                        for m in range(KH):
                            nc.tensor.transpose(
                                tp[:, m * 128:(m + 1) * 128],
                                fT[m][:, b * 128:(b + 1) * 128], identity_bf[:])
                            nc.tensor.matmul(
                                sps[:], fT[m][:, b * 128:(b + 1) * 128],
                                qTb[m][:, rb * 512:(rb + 1) * 512],
                                start=(m == 0), stop=(m == KH - 1))
                        v_chunk = vchp.tile([128, 512], BF16, tag="vch")
                        if b % 2 == 0:
                            nc.vector.tensor_copy(v_chunk[:], tp[:])
                        else:
                            nc.scalar.copy(v_chunk[:], tp[:])

                        mt = maskp.tile([128, 512], BF16, tag="mk")
                        nc.sync.dma_start(
                            mt[:], mask_dram[b * 128:(b + 1) * 128,
                                             rb * 512:(rb + 1) * 512])
                        p32 = pfp.tile([128, 512], F32, tag="p32")
                        nc.vector.tensor_tensor(
                            out=p32[:], in0=sps[:], in1=mt[:],
                            op=mybir.AluOpType.mult)
                        pbf = pbp.tile([128, 512], BF16, tag="pbf")
                        nc.scalar.activation(
                            pbf[:], p32[:], mybir.ActivationFunctionType.Exp,
                            bias=zbias[:], scale=SCALE)

                        # aggregation + per-rs denominator (shares lhsT)
                        for rs in range(4):
                            nc.tensor.matmul(
                                agg[:, rs, :], pbf[:, rs * 128:(rs + 1) * 128],
                                v_chunk[:], start=(b == 0), stop=(b == NB - 1))
                            nc.tensor.matmul(
                                rsum4[:, rs:rs + 1], pbf[:, rs * 128:(rs + 1) * 128],
                                ones_bf[:], start=(b == 0), stop=(b == NB - 1))

                    # epilogue: out rows = agg / rsum + q
                    recip = small.tile([128, 4], F32, tag="recip")
                    nc.vector.reciprocal(recip[:], rsum4[:])
                    o_sbs = []
                    for rs in range(4):
                        o_sb = outsp.tile([128, H], F32, tag=f"o{rs}",
                                          name=f"{pname}o{rb}_{rs}")
                        o_sbs.append(o_sb)
                        if rs % 2 == 0:
                            nc.vector.tensor_scalar(
                                out=o_sb[:], in0=agg[:, rs, :],
                                scalar1=recip[:, rs:rs + 1], scalar2=None,
                                op0=mybir.AluOpType.mult)
                        else:
                            nc.scalar.mul(o_sb[:], agg[:, rs, :],
                                          recip[:, rs:rs + 1])
                    for rs in range(4):
                        qp = ps_tr.tile([128, 512], F32R, tag="tr")
                        for m in range(KH):
                            nc.tensor.transpose(
                                qp[:, m * 128:(m + 1) * 128],
                                qT32[m][:, (rb * 4 + rs) * 128:
                                        (rb * 4 + rs + 1) * 128], identity[:])
                        nc.vector.tensor_tensor(
                            out=o_sbs[rs][:], in0=o_sbs[rs][:], in1=qp[:],
                            op=mybir.AluOpType.add)
                        row0 = out_base + rb * 512 + rs * 128
                        nc.sync.dma_start(out[row0:row0 + 128, :], o_sbs[rs][:])

            # UV direction: q = user rows, feat = item, mask^T = VU_adj cols
            direction(bigA, "A", itemT, userT_blk, W_vT, W_uT, bv_sb, bu_sb,
                      maskA, 0)
            # VU direction: q = item rows, feat = user, mask^T = UV_adj cols
            direction(bigB, "B", userT, itemT_blk, W_uT, W_vT, bu_sb, bv_sb,
                      maskB, RB)

    nc.compile()
    return nc


_NC_CACHE = None
TRACE = False
LAST_RESULT = None


def kernel(user, item, UV_adj, VU_adj, W_u, b_u, W_v, b_v):
    global _NC_CACHE, LAST_RESULT
    user = np.asarray(user, dtype=np.float32)
    item = np.asarray(item, dtype=np.float32)
    UV_adj = np.asarray(UV_adj, dtype=np.float32)
    VU_adj = np.asarray(VU_adj, dtype=np.float32)
    W_u = np.asarray(W_u, dtype=np.float32)
    W_v = np.asarray(W_v, dtype=np.float32)
    b_u = np.asarray(b_u, dtype=np.float32)
    b_v = np.asarray(b_v, dtype=np.float32)

    userT = np.ascontiguousarray(user.T)
    itemT = np.ascontiguousarray(item.T)
    W_uT = np.ascontiguousarray(W_u.T)
    W_vT = np.ascontiguousarray(W_v.T)
    b_u_p = np.ascontiguousarray(b_u.reshape(KH, 128).T)
    b_v_p = np.ascontiguousarray(b_v.reshape(KH, 128).T)
    ident = np.eye(128, dtype=np.float32)

    in_maps = []
    for i in range(NCORES):
        sl = slice(i * RB, (i + 1) * RB)
        in_maps.append({
            "userT": userT,
            "itemT": itemT,
            "userT_blk": np.ascontiguousarray(userT[:, sl]),
            "itemT_blk": np.ascontiguousarray(itemT[:, sl]),
            "maskA": np.ascontiguousarray(VU_adj[:, sl].astype(ml_dtypes.bfloat16)),
            "maskB": np.ascontiguousarray(UV_adj[:, sl].astype(ml_dtypes.bfloat16)),
            "W_uT": W_uT,
            "W_vT": W_vT,
            "b_u_p": b_u_p,
            "b_v_p": b_v_p,
            "ident": ident,
        })

    if _NC_CACHE is None:
        _NC_CACHE = build_nc()
    res = run_bass_kernel_spmd(_NC_CACHE, in_maps, core_ids=list(range(NCORES)),
                               trace=TRACE)
    LAST_RESULT = res
    results = res.results
    learn_user = np.concatenate([results[i]["out"][:RB] for i in range(NCORES)], 0)
    learn_item = np.concatenate([results[i]["out"][RB:] for i in range(NCORES)], 0)
    return (learn_user, learn_item)


if __name__ == "__main__":
    nc = build_nc()
    print("built ok")

